# revision 30
# baseline (speedup 1.0000x reference)
"""Multi-head attention Trainium2 kernel (B=4, T=2048, C=1024, H=16, D=64).

Sharding: 8 cores = 4 batches x 2 head-groups (data parallel on B, tensor
parallel on H). Each core computes attention for 1 batch and 8 heads plus the
partial out-projection for its head rows; the host sums the two partials per
batch (the out-proj "all-reduce"); bias is applied on-device by hg=0 cores.

Device layout notes (per core):
  xT  [C, T]   bf16  x[b] transposed on host
  wq/wk/wv [C, 512] bf16 per-head-group column slices of w_qkv
  wo  [512, C] bf16  row slice of w_out
  bias [1, C]  f32   b_out on hg=0 cores, zeros on hg=1
  y   [T, C]   f32   partial output

  QT/KT: [D,T] per head, two heads packed per 128-partition tile. Scores
  S^T[k,q] matmuls alternate the two heads (disjoint PE row groups) so
  consecutive matmuls can overlap in the array. exp() runs on ScalarE
  straight out of PSUM (logits bounded, no max subtraction needed) into an
  interleaved expS ring in SBUF. V is kept natural [T,D] with an appended
  ones column so the M=65 PV matmul produces O^T (rows 0..63) and the
  softmax denominators (row 64) in one pass. Reciprocal via fast DVE approx
  (input must sit at partition 0), partition-broadcast on GpSimd, then the
  out-projection consumes Theta^T directly as the stationary operand.
"""

import numpy as np
import ml_dtypes

import concourse.bacc as bacc
import concourse.mybir as mybir
import concourse.tile as tile
from concourse.bass_utils import run_bass_kernel_spmd

B, T, C, H, D = 4, 2048, 1024, 16, 64
HPC = 8          # heads per core
PAIRS = HPC // 2
CT = C // 128    # 8 contraction tiles for projections
TT = T // 128    # 16 t-tiles (also k-tiles of attention)
QC = T // 512    # 4 query chunks
JC = C // 512    # 2 out-proj column chunks
BF16 = mybir.dt.bfloat16
F32 = mybir.dt.float32
EXP = mybir.ActivationFunctionType.Exp

_CACHED_NC = None


def _emit(nc, tc, xT_d, wq_d, wk_d, wv_d, wo_d, bias_d, y_d):
    import contextlib
    with contextlib.ExitStack() as ctx:
        persist = ctx.enter_context(tc.tile_pool(name="persist", bufs=1))
        work = ctx.enter_context(tc.tile_pool(name="work", bufs=2))
        spsum = ctx.enter_context(tc.tile_pool(name="spsum", bufs=2, space="PSUM"))
        apsum = ctx.enter_context(tc.tile_pool(name="apsum", bufs=2, space="PSUM"))
        ppsum = ctx.enter_context(tc.tile_pool(name="ppsum", bufs=2, space="PSUM"))

        # ---- static loads ----
        # emission order = DMA queue order: interleave per-ctile so the first
        # projection chains can start while later tiles stream in; split
        # across two queue engines (sync + gpsimd)
        xT_sb = []
        w_sb = {}
        for i in range(CT):
            t = persist.tile([128, T], BF16, tag=f"xT{i}", name=f"xT{i}")
            (nc.sync if i % 2 == 0 else nc.scalar).dma_start(
                out=t, in_=xT_d[i * 128:(i + 1) * 128, :])
            xT_sb.append(t)
            for wname, wd in (("wk", wk_d), ("wq", wq_d)):
                t = persist.tile([128, 512], BF16, tag=f"{wname}{i}", name=f"{wname}{i}")
                nc.gpsimd.dma_start(out=t, in_=wd[i * 128:(i + 1) * 128, :])
                w_sb[(wname, i)] = t
        for i in range(CT):
            t = persist.tile([128, 512], BF16, tag=f"wv{i}", name=f"wv{i}")
            nc.gpsimd.dma_start(out=t, in_=wv_d[i * 128:(i + 1) * 128, :])
            w_sb[("wv", i)] = t
        wo_sb = []
        for i in range(4):
            t = persist.tile([128, C], BF16, tag=f"wo{i}", name=f"wo{i}")
            nc.gpsimd.dma_start(out=t, in_=wo_d[i * 128:(i + 1) * 128, :])
            wo_sb.append(t)
        bias_sb = persist.tile([1, C], F32, tag="bias", name="bias")
        nc.gpsimd.dma_start(out=bias_sb, in_=bias_d[0:1, :])
        bias_bc = persist.tile([128, C], F32, tag="bias_bc", name="bias_bc")
        nc.gpsimd.partition_broadcast(bias_bc, bias_sb)

        # V natural [T, 512] + ones column per head -> Vaug tiles [128, 8, 65]
        vaug = [persist.tile([128, HPC, D + 1], BF16, tag=f"vaug{tt}", name=f"vaug{tt}")
                for tt in range(TT)]

        def v_chunk(tt):
            vt = vaug[tt]
            ps = ppsum.tile([128, 512], F32, tag="proj", name="vps")
            for c in range(CT):
                nc.tensor.matmul(ps, lhsT=xT_sb[c][:, tt * 128:(tt + 1) * 128],
                                 rhs=w_sb[("wv", c)], start=(c == 0), stop=(c == CT - 1))
            nc.vector.tensor_copy(
                out=vt[:, :, 0:D],
                in_=ps.rearrange("p (h d) -> p h d", h=HPC))
            nc.vector.memset(vt[:, :, D:D + 1], 1.0)

        # Q^T / K^T tiles [128 = 2 heads x 64, T]; filled lazily per pair so
        # later pairs' projections overlap earlier pairs' ACT-bound attention
        qt_sb = [persist.tile([128, T], BF16, tag=f"qt{p}", name=f"qt{p}")
                 for p in range(PAIRS)]
        kt_sb = [persist.tile([128, T], BF16, tag=f"kt{p}", name=f"kt{p}")
                 for p in range(PAIRS)]

        def project_chunk(p, dst, wname, qc):
            ps = ppsum.tile([128, 512], F32, tag="proj", name="qkps")
            for c in range(CT):
                nc.tensor.matmul(
                    ps,
                    lhsT=w_sb[(wname, c)][:, p * 128:(p + 1) * 128],
                    rhs=xT_sb[c][:, qc * 512:(qc + 1) * 512],
                    start=(c == 0), stop=(c == CT - 1))
            nc.vector.tensor_copy(out=dst[:, qc * 512:(qc + 1) * 512], in_=ps)

        # pair-0 Q/K upfront (K first: scores need all K^T chunks, Q^T JIT);
        # V projection is woven into (p0, qc0)'s score loop
        for qc in range(QC):
            project_chunk(0, kt_sb[0], "wk", qc)
        for qc in range(QC):
            project_chunk(0, qt_sb[0], "wq", qc)

        # ---- attention ----
        # expS ring: interleaved [h0 kt | h1 kt] units of 512, RING=40 units
        # (1.25 sections) so exp of section s+1 can run ahead while PV of
        # section s drains; subtile deps handle the wrap-around reuse.
        RING = 40
        exps = persist.tile([128, RING * 512], BF16, tag="expS", name="expS")
        tht_sb = [persist.tile([128, T], BF16, tag=f"tht{p}", name=f"tht{p}")
                  for p in range(PAIRS)]
        # filler work emitted after each (p, qc) section: the next pair's
        # projections (and, for p0/qc0, the V projection) fill PE bubbles
        # while the current attention chunk is ACT-paced
        # just-in-time projection fillers: each entry (pair, wname, chunk) is
        # emitted after section (p, qc); K chunks precede Q chunks since
        # scores(p, qc0) read all of K^T but only one Q^T chunk
        fillers = {
            (0, 1): [(1, "wk", 0), (1, "wk", 1), (1, "wk", 2)],
            (0, 2): [(1, "wk", 3), (1, "wq", 0), (1, "wq", 1)],
            (0, 3): [(1, "wq", 2), (1, "wq", 3)],
            (1, 0): [(2, "wk", 0), (2, "wk", 1)],
            (1, 1): [(2, "wk", 2), (2, "wk", 3)],
            (1, 2): [(2, "wq", 0), (2, "wq", 1)],
            (1, 3): [(2, "wq", 2), (2, "wq", 3)],
            (2, 0): [(3, "wk", 0), (3, "wk", 1)],
            (2, 1): [(3, "wk", 2), (3, "wk", 3)],
            (2, 2): [(3, "wq", 0), (3, "wq", 1)],
            (2, 3): [(3, "wq", 2), (3, "wq", 3)],
        }

        def out_proj_group(tt):
            ysb = work.tile([128, C], F32, tag="ysb", bufs=3, name="ysb")
            for jc in range(JC):
                jsl = slice(jc * 512, (jc + 1) * 512)
                # alternate accumulator pools: ppsum is mostly idle during
                # the last pair (few projection fillers left)
                pool, tg = ((apsum, "acc") if (tt + jc) % 2 == 0
                            else (ppsum, "proj"))
                yps = pool.tile([128, 512], F32, tag=tg, name="yps")
                for pp in range(PAIRS):
                    nc.tensor.matmul(
                        yps, lhsT=tht_sb[pp][:, tt * 128:(tt + 1) * 128],
                        rhs=wo_sb[pp][:, jsl],
                        start=(pp == 0), stop=(pp == PAIRS - 1))
                nc.vector.tensor_add(out=ysb[:, jsl], in0=yps,
                                     in1=bias_bc[:, jsl])
            eng = nc.sync if tt % 2 == 0 else nc.gpsimd
            eng.dma_start(out=y_d[tt * 128:(tt + 1) * 128, :], in_=ysb)

        ring_base = 0
        for p in range(PAIRS):
            for qc in range(QC):
                qsl = slice(qc * 512, (qc + 1) * 512)

                def unit(kt, lh):
                    u = (ring_base + 2 * kt + lh) % RING
                    return slice(u * 512, (u + 1) * 512)

                # scores + exp: adjacent matmuls alternate PE row groups
                # (h0 rows 0-63, h1 rows 64-127) so they can overlap
                for kt in range(TT):
                    ps = spsum.tile([128, 1024], F32, tag="mm", name="sps")
                    for lh in range(2):
                        hsl = slice(lh * 64, (lh + 1) * 64)
                        nc.tensor.matmul(
                            ps[:, lh * 512:(lh + 1) * 512],
                            lhsT=kt_sb[p][hsl, kt * 128:(kt + 1) * 128],
                            rhs=qt_sb[p][hsl, qsl],
                            start=True, stop=True)
                    u0 = (ring_base + 2 * kt) % RING
                    nc.scalar.activation(
                        out=exps[:, u0 * 512:(u0 + 2) * 512],
                        in_=ps, func=EXP, scale=0.125)
                    if p == 0 and qc == 0:
                        # V projection woven into the exp-paced score loop
                        v_chunk(kt)
                # PV: both heads' accumulation chains interleaved so ring
                # units free in kt order and exp of the next section can
                # overwrite them while these chains drain
                ops = [apsum.tile([D + 1, 512], F32, tag="acc", name=f"ops{lh}")
                       for lh in range(2)]
                for kt in range(TT):
                    for lh in range(2):
                        nc.tensor.matmul(
                            ops[lh], lhsT=vaug[kt][:, 2 * p + lh, :],
                            rhs=exps[:, unit(kt, lh)],
                            start=(kt == 0), stop=(kt == TT - 1))
                for lh in range(2):
                    # copy sums to partition 0 first: the custom-DVE fast
                    # reciprocal misreads partition-shifted inputs
                    ssb = work.tile([1, 512], F32, tag="ssb", name="ssb")
                    nc.vector.tensor_copy(out=ssb, in_=ops[lh][D:D + 1, :])
                    rsb = work.tile([1, 512], F32, tag="rsb", name="rsb")
                    nc.vector.reciprocal_approx_fast(out=rsb, in_=ssb)
                    rbc = work.tile([64, 512], F32, tag="rbc", name="rbc")
                    nc.gpsimd.partition_broadcast(rbc, rsb)
                    nc.vector.tensor_mul(
                        out=tht_sb[p][lh * 64:(lh + 1) * 64, qsl],
                        in0=ops[lh][0:D, :], in1=rbc)
                ring_base = (ring_base + 2 * TT) % RING
                for fp, wname, fqc in fillers.get((p, qc), []):
                    dst = qt_sb[fp] if wname == "wq" else kt_sb[fp]
                    project_chunk(fp, dst, wname, fqc)

                # out-projection (bias via DVE add, not PE): t-tiles of the
                # PREVIOUS qc chunk, so C never delays this section's scores
                if p == PAIRS - 1 and qc >= 1:
                    for tt in range(4 * (qc - 1), 4 * qc):
                        out_proj_group(tt)

        for tt in range(4 * (QC - 1), 4 * QC):
            out_proj_group(tt)


def _build():
    nc = bacc.Bacc("TRN2", target_bir_lowering=False)
    xT_d = nc.dram_tensor("xT", [C, T], BF16, kind="ExternalInput")
    wq_d = nc.dram_tensor("wq", [C, 512], BF16, kind="ExternalInput")
    wk_d = nc.dram_tensor("wk", [C, 512], BF16, kind="ExternalInput")
    wv_d = nc.dram_tensor("wv", [C, 512], BF16, kind="ExternalInput")
    wo_d = nc.dram_tensor("wo", [512, C], BF16, kind="ExternalInput")
    bias_d = nc.dram_tensor("bias", [1, C], F32, kind="ExternalInput")
    y_d = nc.dram_tensor("y", [T, C], F32, kind="ExternalOutput")
    with tile.TileContext(nc) as tc:
        _emit(nc, tc, xT_d, wq_d, wk_d, wv_d, wo_d, bias_d, y_d)
    if not nc.is_finalized():
        nc.finalize()
    return nc


def get_nc():
    global _CACHED_NC
    if _CACHED_NC is None:
        _CACHED_NC = _build()
    return _CACHED_NC


def make_in_maps(x, w_qkv, w_out, b_out):
    bf = ml_dtypes.bfloat16
    x = np.asarray(x, dtype=np.float32)
    w_qkv = np.asarray(w_qkv, dtype=np.float32)
    w_out = np.asarray(w_out, dtype=np.float32)
    b_out = np.asarray(b_out, dtype=np.float32)
    in_maps = []
    for core in range(8):
        b, hg = core // 2, core % 2
        cs = slice(hg * 512, (hg + 1) * 512)
        bias = b_out if hg == 0 else np.zeros_like(b_out)
        in_maps.append({
            "xT": np.ascontiguousarray(x[b].T).astype(bf),
            "wq": np.ascontiguousarray(w_qkv[:, 0 * C:][:, cs]).astype(bf),
            "wk": np.ascontiguousarray(w_qkv[:, 1 * C:][:, cs]).astype(bf),
            "wv": np.ascontiguousarray(w_qkv[:, 2 * C:][:, cs]).astype(bf),
            "wo": np.ascontiguousarray(w_out[cs, :]).astype(bf),
            "bias": np.ascontiguousarray(bias.reshape(1, C), dtype=np.float32),
        })
    return in_maps


def _ensure_ntff_hook():
    """Register the axon NTFF profile hook if the container's antenv lacks
    axon_hooks (test/profiling use only; never needed for plain kernel())."""
    import sys
    import types
    try:
        from antenv import axon_hooks  # noqa: F401
    except ImportError:
        mod = types.ModuleType("antenv.axon_hooks")
        mod._hook = None

        def set_axon_ntff_profile_hook(hook, _m=mod):
            _m._hook = hook

        def get_axon_ntff_profile_hook(_m=mod):
            return _m._hook

        mod.set_axon_ntff_profile_hook = set_axon_ntff_profile_hook
        mod.get_axon_ntff_profile_hook = get_axon_ntff_profile_hook
        sys.modules["antenv.axon_hooks"] = mod
        import antenv
        antenv.axon_hooks = mod
    import antenv.axon_hooks as ah
    if ah.get_axon_ntff_profile_hook() is None:
        from trn_agent_boot.trn_boot import _ntff_profile_via_ctypes
        ah.set_axon_ntff_profile_hook(
            _ntff_profile_via_ctypes("/opt/axon/libaxon_pjrt.so"))


def kernel(x, w_qkv, w_out, b_out, _trace=False, _trace_kwargs=None):
    nc = get_nc()
    in_maps = make_in_maps(x, w_qkv, w_out, b_out)
    kwargs = {}
    if _trace:
        try:
            _ensure_ntff_hook()
        except Exception as e:
            print(f"NTFF hook setup failed ({e}); running without trace")
        else:
            kwargs.update(trace=True, **(_trace_kwargs or {}))
    res = run_bass_kernel_spmd(nc, in_maps, core_ids=list(range(8)), **kwargs)
    out = np.empty((B, T, C), dtype=np.float32)
    for b in range(B):
        out[b] = res.results[2 * b]["y"] + res.results[2 * b + 1]["y"]
    if _trace:
        return out, res
    return out


# revision 31
# speedup vs baseline: 1.0077x; 1.0077x over previous
"""Multi-head attention Trainium2 kernel (B=4, T=2048, C=1024, H=16, D=64).

Sharding: 8 cores = 4 batches x 2 head-groups (data parallel on B, tensor
parallel on H). Each core computes attention for 1 batch and 8 heads plus the
partial out-projection for its head rows; the host sums the two partials per
batch (the out-proj "all-reduce"); bias is applied on-device by hg=0 cores.

Device layout notes (per core):
  xT  [C, T]   bf16  x[b] transposed on host
  wq/wk/wv [C, 512] bf16 per-head-group column slices of w_qkv
  wo  [512, C] bf16  row slice of w_out
  bias [1, C]  f32   b_out on hg=0 cores, zeros on hg=1
  y   [T, C]   f32   partial output

  QT/KT: [D,T] per head, two heads packed per 128-partition tile. Scores
  S^T[k,q] matmuls alternate the two heads (disjoint PE row groups) so
  consecutive matmuls can overlap in the array. exp() runs on ScalarE
  straight out of PSUM (logits bounded, no max subtraction needed) into an
  interleaved expS ring in SBUF. V is kept natural [T,D] with an appended
  ones column so the M=65 PV matmul produces O^T (rows 0..63) and the
  softmax denominators (row 64) in one pass. Reciprocal via fast DVE approx
  (input must sit at partition 0), partition-broadcast on GpSimd, then the
  out-projection consumes Theta^T directly as the stationary operand.
"""

import numpy as np
import ml_dtypes

import concourse.bacc as bacc
import concourse.mybir as mybir
import concourse.tile as tile
from concourse.bass_utils import run_bass_kernel_spmd

B, T, C, H, D = 4, 2048, 1024, 16, 64
HPC = 8          # heads per core
PAIRS = HPC // 2
CT = C // 128    # 8 contraction tiles for projections
TT = T // 128    # 16 t-tiles (also k-tiles of attention)
QC = T // 512    # 4 query chunks
JC = C // 512    # 2 out-proj column chunks
BF16 = mybir.dt.bfloat16
F32 = mybir.dt.float32
EXP = mybir.ActivationFunctionType.Exp

_CACHED_NC = None


def _emit(nc, tc, xT_d, wq_d, wk_d, wv_d, wo_d, bias_d, y_d):
    import contextlib
    with contextlib.ExitStack() as ctx:
        persist = ctx.enter_context(tc.tile_pool(name="persist", bufs=1))
        work = ctx.enter_context(tc.tile_pool(name="work", bufs=2))
        spsum = ctx.enter_context(tc.tile_pool(name="spsum", bufs=2, space="PSUM"))
        apsum = ctx.enter_context(tc.tile_pool(name="apsum", bufs=2, space="PSUM"))
        ppsum = ctx.enter_context(tc.tile_pool(name="ppsum", bufs=2, space="PSUM"))

        # ---- static loads ----
        # emission order = DMA queue order: interleave per-ctile so the first
        # projection chains can start while later tiles stream in; split
        # across two queue engines (sync + gpsimd)
        xT_sb = []
        w_sb = {}
        for i in range(CT):
            t = persist.tile([128, T], BF16, tag=f"xT{i}", name=f"xT{i}")
            (nc.sync if i % 2 == 0 else nc.scalar).dma_start(
                out=t, in_=xT_d[i * 128:(i + 1) * 128, :])
            xT_sb.append(t)
            for wname, wd in (("wk", wk_d), ("wq", wq_d)):
                t = persist.tile([128, 512], BF16, tag=f"{wname}{i}", name=f"{wname}{i}")
                nc.gpsimd.dma_start(out=t, in_=wd[i * 128:(i + 1) * 128, :])
                w_sb[(wname, i)] = t
        for i in range(CT):
            t = persist.tile([128, 512], BF16, tag=f"wv{i}", name=f"wv{i}")
            nc.gpsimd.dma_start(out=t, in_=wv_d[i * 128:(i + 1) * 128, :])
            w_sb[("wv", i)] = t
        wo_sb = []
        for i in range(4):
            t = persist.tile([128, C], BF16, tag=f"wo{i}", name=f"wo{i}")
            nc.gpsimd.dma_start(out=t, in_=wo_d[i * 128:(i + 1) * 128, :])
            wo_sb.append(t)
        bias_sb = persist.tile([1, C], F32, tag="bias", name="bias")
        nc.gpsimd.dma_start(out=bias_sb, in_=bias_d[0:1, :])
        bias_bc = persist.tile([128, C], F32, tag="bias_bc", name="bias_bc")
        nc.gpsimd.partition_broadcast(bias_bc, bias_sb)

        # V natural [T, 512] + ones column per head -> Vaug tiles [128, 8, 65]
        vaug = [persist.tile([128, HPC, D + 1], BF16, tag=f"vaug{tt}", name=f"vaug{tt}")
                for tt in range(TT)]

        def v_chunk(tt):
            vt = vaug[tt]
            ps = ppsum.tile([128, 512], F32, tag="proj", name="vps")
            for c in range(CT):
                nc.tensor.matmul(ps, lhsT=xT_sb[c][:, tt * 128:(tt + 1) * 128],
                                 rhs=w_sb[("wv", c)], start=(c == 0), stop=(c == CT - 1))
            nc.vector.tensor_copy(
                out=vt[:, :, 0:D],
                in_=ps.rearrange("p (h d) -> p h d", h=HPC))
            nc.vector.memset(vt[:, :, D:D + 1], 1.0)

        # Q^T / K^T tiles [128 = 2 heads x 64, T]; filled lazily per pair so
        # later pairs' projections overlap earlier pairs' ACT-bound attention
        qt_sb = [persist.tile([128, T], BF16, tag=f"qt{p}", name=f"qt{p}")
                 for p in range(PAIRS)]
        kt_sb = [persist.tile([128, T], BF16, tag=f"kt{p}", name=f"kt{p}")
                 for p in range(PAIRS)]

        def project_chunk(p, dst, wname, qc):
            ps = ppsum.tile([128, 512], F32, tag="proj", name="qkps")
            for c in range(CT):
                nc.tensor.matmul(
                    ps,
                    lhsT=w_sb[(wname, c)][:, p * 128:(p + 1) * 128],
                    rhs=xT_sb[c][:, qc * 512:(qc + 1) * 512],
                    start=(c == 0), stop=(c == CT - 1))
            nc.vector.tensor_copy(out=dst[:, qc * 512:(qc + 1) * 512], in_=ps)

        # pair-0 Q/K upfront (K first: scores need all K^T chunks, Q^T JIT);
        # V projection is woven into (p0, qc0)'s score loop
        for qc in range(QC):
            project_chunk(0, kt_sb[0], "wk", qc)
        for qc in range(QC):
            project_chunk(0, qt_sb[0], "wq", qc)

        # ---- attention ----
        # expS ring: interleaved [h0 kt | h1 kt] units of 512, RING=40 units
        # (1.25 sections) so exp of section s+1 can run ahead while PV of
        # section s drains; subtile deps handle the wrap-around reuse.
        RING = 40
        exps = persist.tile([128, RING * 512], BF16, tag="expS", name="expS")
        tht_sb = [persist.tile([128, T], BF16, tag=f"tht{p}", name=f"tht{p}")
                  for p in range(PAIRS)]
        # filler work emitted after each (p, qc) section: the next pair's
        # projections (and, for p0/qc0, the V projection) fill PE bubbles
        # while the current attention chunk is ACT-paced
        # just-in-time projection fillers: each entry (pair, wname, chunk) is
        # emitted after section (p, qc); K chunks precede Q chunks since
        # scores(p, qc0) read all of K^T but only one Q^T chunk
        fillers = {
            (0, 1): [(1, "wk", 0), (1, "wk", 1), (1, "wk", 2)],
            (0, 2): [(1, "wk", 3), (1, "wq", 0), (1, "wq", 1)],
            (0, 3): [(1, "wq", 2), (1, "wq", 3)],
            (1, 0): [(2, "wk", 0), (2, "wk", 1)],
            (1, 1): [(2, "wk", 2), (2, "wk", 3)],
            (1, 2): [(2, "wq", 0), (2, "wq", 1)],
            (1, 3): [(2, "wq", 2), (2, "wq", 3)],
            (2, 0): [(3, "wk", 0), (3, "wk", 1)],
            (2, 1): [(3, "wk", 2), (3, "wk", 3)],
            (2, 2): [(3, "wq", 0), (3, "wq", 1)],
            (2, 3): [(3, "wq", 2), (3, "wq", 3)],
        }

        def out_proj_group(tt):
            ysb = work.tile([128, C], F32, tag="ysb", bufs=3, name="ysb")
            for jc in range(JC):
                jsl = slice(jc * 512, (jc + 1) * 512)
                # alternate accumulator pools: ppsum is mostly idle during
                # the last pair (few projection fillers left)
                pool, tg = ((apsum, "acc") if (tt + jc) % 2 == 0
                            else (ppsum, "proj"))
                yps = pool.tile([128, 512], F32, tag=tg, name="yps")
                for pp in range(PAIRS):
                    nc.tensor.matmul(
                        yps, lhsT=tht_sb[pp][:, tt * 128:(tt + 1) * 128],
                        rhs=wo_sb[pp][:, jsl],
                        start=(pp == 0), stop=(pp == PAIRS - 1))
                nc.vector.tensor_add(out=ysb[:, jsl], in0=yps,
                                     in1=bias_bc[:, jsl])
            eng = nc.sync if tt % 2 == 0 else nc.gpsimd
            eng.dma_start(out=y_d[tt * 128:(tt + 1) * 128, :], in_=ysb)

        ring_base = 0
        for p in range(PAIRS):
            for qc in range(QC):
                qsl = slice(qc * 512, (qc + 1) * 512)

                def unit(kt, lh):
                    u = (ring_base + 2 * kt + lh) % RING
                    return slice(u * 512, (u + 1) * 512)

                # scores + exp: adjacent matmuls alternate PE row groups
                # (h0 rows 0-63, h1 rows 64-127) so they can overlap
                for kt in range(TT):
                    ps = spsum.tile([128, 1024], F32, tag="mm", name="sps")
                    for lh in range(2):
                        hsl = slice(lh * 64, (lh + 1) * 64)
                        nc.tensor.matmul(
                            ps[:, lh * 512:(lh + 1) * 512],
                            lhsT=kt_sb[p][hsl, kt * 128:(kt + 1) * 128],
                            rhs=qt_sb[p][hsl, qsl],
                            start=True, stop=True)
                    u0 = (ring_base + 2 * kt) % RING
                    nc.scalar.activation(
                        out=exps[:, u0 * 512:(u0 + 2) * 512],
                        in_=ps, func=EXP, scale=0.125)
                    if p == 0 and qc == 0 and kt < 10:
                        # V projection woven into the exp-paced score loops of
                        # the first two sections (PV of section 0 waits on the
                        # late tiles; the expS ring absorbs the skew)
                        v_chunk(kt)
                    elif p == 0 and qc == 1 and kt < 6:
                        v_chunk(10 + kt)
                # PV: both heads' accumulation chains interleaved so ring
                # units free in kt order and exp of the next section can
                # overwrite them while these chains drain
                ops = [apsum.tile([D + 1, 512], F32, tag="acc", name=f"ops{lh}")
                       for lh in range(2)]
                for kt in range(TT):
                    for lh in range(2):
                        nc.tensor.matmul(
                            ops[lh], lhsT=vaug[kt][:, 2 * p + lh, :],
                            rhs=exps[:, unit(kt, lh)],
                            start=(kt == 0), stop=(kt == TT - 1))
                for lh in range(2):
                    # copy sums to partition 0 first: the custom-DVE fast
                    # reciprocal misreads partition-shifted inputs
                    ssb = work.tile([1, 512], F32, tag="ssb", name="ssb")
                    nc.vector.tensor_copy(out=ssb, in_=ops[lh][D:D + 1, :])
                    rsb = work.tile([1, 512], F32, tag="rsb", name="rsb")
                    nc.vector.reciprocal_approx_fast(out=rsb, in_=ssb)
                    rbc = work.tile([64, 512], F32, tag="rbc", name="rbc")
                    nc.gpsimd.partition_broadcast(rbc, rsb)
                    nc.vector.tensor_mul(
                        out=tht_sb[p][lh * 64:(lh + 1) * 64, qsl],
                        in0=ops[lh][0:D, :], in1=rbc)
                ring_base = (ring_base + 2 * TT) % RING
                for fp, wname, fqc in fillers.get((p, qc), []):
                    dst = qt_sb[fp] if wname == "wq" else kt_sb[fp]
                    project_chunk(fp, dst, wname, fqc)

                # out-projection (bias via DVE add, not PE): t-tiles of the
                # PREVIOUS qc chunk, so C never delays this section's scores
                if p == PAIRS - 1 and qc >= 1:
                    for tt in range(4 * (qc - 1), 4 * qc):
                        out_proj_group(tt)

        for tt in range(4 * (QC - 1), 4 * QC):
            out_proj_group(tt)


def _build():
    nc = bacc.Bacc("TRN2", target_bir_lowering=False)
    xT_d = nc.dram_tensor("xT", [C, T], BF16, kind="ExternalInput")
    wq_d = nc.dram_tensor("wq", [C, 512], BF16, kind="ExternalInput")
    wk_d = nc.dram_tensor("wk", [C, 512], BF16, kind="ExternalInput")
    wv_d = nc.dram_tensor("wv", [C, 512], BF16, kind="ExternalInput")
    wo_d = nc.dram_tensor("wo", [512, C], BF16, kind="ExternalInput")
    bias_d = nc.dram_tensor("bias", [1, C], F32, kind="ExternalInput")
    y_d = nc.dram_tensor("y", [T, C], F32, kind="ExternalOutput")
    with tile.TileContext(nc) as tc:
        _emit(nc, tc, xT_d, wq_d, wk_d, wv_d, wo_d, bias_d, y_d)
    if not nc.is_finalized():
        nc.finalize()
    return nc


def get_nc():
    global _CACHED_NC
    if _CACHED_NC is None:
        _CACHED_NC = _build()
    return _CACHED_NC


def make_in_maps(x, w_qkv, w_out, b_out):
    bf = ml_dtypes.bfloat16
    x = np.asarray(x, dtype=np.float32)
    w_qkv = np.asarray(w_qkv, dtype=np.float32)
    w_out = np.asarray(w_out, dtype=np.float32)
    b_out = np.asarray(b_out, dtype=np.float32)
    in_maps = []
    for core in range(8):
        b, hg = core // 2, core % 2
        cs = slice(hg * 512, (hg + 1) * 512)
        bias = b_out if hg == 0 else np.zeros_like(b_out)
        in_maps.append({
            "xT": np.ascontiguousarray(x[b].T).astype(bf),
            "wq": np.ascontiguousarray(w_qkv[:, 0 * C:][:, cs]).astype(bf),
            "wk": np.ascontiguousarray(w_qkv[:, 1 * C:][:, cs]).astype(bf),
            "wv": np.ascontiguousarray(w_qkv[:, 2 * C:][:, cs]).astype(bf),
            "wo": np.ascontiguousarray(w_out[cs, :]).astype(bf),
            "bias": np.ascontiguousarray(bias.reshape(1, C), dtype=np.float32),
        })
    return in_maps


def _ensure_ntff_hook():
    """Register the axon NTFF profile hook if the container's antenv lacks
    axon_hooks (test/profiling use only; never needed for plain kernel())."""
    import sys
    import types
    try:
        from antenv import axon_hooks  # noqa: F401
    except ImportError:
        mod = types.ModuleType("antenv.axon_hooks")
        mod._hook = None

        def set_axon_ntff_profile_hook(hook, _m=mod):
            _m._hook = hook

        def get_axon_ntff_profile_hook(_m=mod):
            return _m._hook

        mod.set_axon_ntff_profile_hook = set_axon_ntff_profile_hook
        mod.get_axon_ntff_profile_hook = get_axon_ntff_profile_hook
        sys.modules["antenv.axon_hooks"] = mod
        import antenv
        antenv.axon_hooks = mod
    import antenv.axon_hooks as ah
    if ah.get_axon_ntff_profile_hook() is None:
        from trn_agent_boot.trn_boot import _ntff_profile_via_ctypes
        ah.set_axon_ntff_profile_hook(
            _ntff_profile_via_ctypes("/opt/axon/libaxon_pjrt.so"))


def kernel(x, w_qkv, w_out, b_out, _trace=False, _trace_kwargs=None):
    nc = get_nc()
    in_maps = make_in_maps(x, w_qkv, w_out, b_out)
    kwargs = {}
    if _trace:
        try:
            _ensure_ntff_hook()
        except Exception as e:
            print(f"NTFF hook setup failed ({e}); running without trace")
        else:
            kwargs.update(trace=True, **(_trace_kwargs or {}))
    res = run_bass_kernel_spmd(nc, in_maps, core_ids=list(range(8)), **kwargs)
    out = np.empty((B, T, C), dtype=np.float32)
    for b in range(B):
        out[b] = res.results[2 * b]["y"] + res.results[2 * b + 1]["y"]
    if _trace:
        return out, res
    return out
